# revision 54
# baseline (speedup 1.0000x reference)
"""Trainium2 Bass kernel for grouped-query causal self-attention (v2).

Problem shapes (hardcoded): x [8,1024,1024] f32, W_attn [6144,1024] f32,
W_proj [1024,4096] f32. 16 heads, head_dim 64, 4 query sets sharing one K/V.

Sharding: data parallel over batch - one batch element per NeuronCore (8
cores), no collectives.

Per-core algorithm, attention in transposed [feature, token] layout (no
on-device transposes; x^T prepared on the host). bf16 matmul operands with
fp32 PSUM accumulation (fp8 DoubleRow was tried and is numerically
infeasible here: attention output y is ~8x smaller than v, so fp8's ~4%
element error survives averaging as ~5% relative output error > 2e-2).

The kernel is organized so the in-order PE queue never sits behind a
dependency-stalled instruction:
  - Attention per (set g, head-pair hp): heads 2hp/2hp+1 share kt/qt tiles
    (rows 0-63 / 64-127). Per 128-kpos block k2: two 64-contraction score
    matmuls write one 2-bank PSUM tile; ONE scalar-engine exp instruction
    [128, 2(head), w] produces P; causal diagonal triangles are zeroed by
    gpsimd affine_select (the PE never computes a mask); AV matmuls
    (V augmented with a ones column -> PSUM row 64 = softmax denominator)
    lag one k-block behind the scores.
  - The attention phase is exp-rate-bound, so independent matmul work is
    interleaved between attention matmuls as fine-grained "filler" closures
    sitting AHEAD of stall points in the PE queue: K / Q projections are
    self-feeding (f-tile hp+1 of the running set's weights is built during
    block hp, one block before its first reader), and the previous set's
    output projection runs as 8 interleaved pieces. The last set runs
    ftl 0-5 of its own projection inside its final two attention blocks,
    leaving only a 2-ftl tail after the last normalize.
  - Denominators: yp rows 64 are DMA-gathered per two head-pairs into a
    [4, 1024] tile, inverted with reciprocal_approx_fast, bounced through
    DRAM for a partition-broadcast read, and applied as bf16 2x-rate DVE
    multiplies into yt (double-buffered across sets so the next set's
    attention can overwrite while the previous projection still reads).
  - DMAs are spread across the sync and activation HWDGE queues so weight
    prefetches never queue behind the normalize bounce traffic.
out = combined @ W_proj^T accumulated over sets in SBUF (bf16), written as
bf16 and upcast on the host.
"""

import math

import ml_dtypes
import numpy as np

import concourse.bacc as bacc
import concourse.bass as bass
import concourse.mybir as mybir
import concourse.tile as tile
from concourse.bass_utils import run_bass_kernel_spmd

BF16 = ml_dtypes.bfloat16
FP8 = ml_dtypes.float8_e4m3

B, T, C = 8, 1024, 1024
NH, HD, NQS = 16, 64, 4
NHP = NH // 2
SCALE = 1.0 / math.sqrt(HD)
NT = T // 128
NCH = C // 128
KOFF = NQS * C  # 4096: K rows in W_attn
VOFF = (NQS + 1) * C  # 5120: V rows in W_attn

FP8_QKV = False  # qkv projection via fp8 DoubleRow matmuls
FP8_AV = False  # P/V in fp8, AV via DoubleRow matmuls
XS = 8.0  # host scale on x before fp8 cast
WS = 4.0  # host scale on W_attn before fp8 cast

_CACHE = {}
LAST = {}  # exec_time_ns etc for test harness


def _build():
    f32 = mybir.dt.float32
    bf16 = mybir.dt.bfloat16
    fp8 = mybir.dt.float8e4
    EXP = mybir.ActivationFunctionType.Exp
    DR = mybir.MatmulPerfMode.DoubleRow
    pdt = fp8 if FP8_AV else bf16
    qk_scale = SCALE / ((XS * WS) ** 2 if FP8_QKV else 1.0)
    exp_bias = -2.0 if FP8_AV else 0.0

    nc = bacc.Bacc()
    if FP8_QKV:
        xTD = nc.declare_dram_parameter("xT", [4, 128, 2, T], fp8, isOutput=False)
        waTD = nc.declare_dram_parameter("waT", [4, 128, 2, 6 * C], fp8, isOutput=False)
    else:
        xTD = nc.declare_dram_parameter("xT", [C, T], bf16, isOutput=False)
        waTD = nc.declare_dram_parameter("waT", [C, 6 * C], bf16, isOutput=False)
    wpTD = nc.declare_dram_parameter("wpT", [NQS * C, C], bf16, isOutput=False)
    out = nc.declare_dram_parameter("out", [T, C], bf16, isOutput=True)
    # reciprocal rows bounced through DRAM for partition-broadcast reads
    rscratch = nc.dram_tensor("rscratch", [NQS * 16, 1024], bf16)

    with tile.TileContext(nc) as tc:
        with (
            tc.tile_pool(name="res", bufs=1) as res,
            tc.tile_pool(name="wa", bufs=18) as wa_pool,
            tc.tile_pool(name="wp", bufs=8) as wp_pool,
            tc.tile_pool(name="pt", bufs=4) as pt_pool,
            tc.tile_pool(name="bc", bufs=3) as bc_pool,
            tc.tile_pool(name="yst", bufs=6) as yst_pool,
            tc.tile_pool(name="spq", bufs=2, space="PSUM") as spq_pool,
            tc.tile_pool(name="ypp", bufs=1, space="PSUM") as yp_pool,
        ):
            if FP8_QKV:
                xt = [res.tile([128, 2, T], fp8, tag=f"xt{i}", name=f"xt{i}") for i in range(4)]
            else:
                xt = [res.tile([128, T], bf16, tag=f"xt{i}", name=f"xt{i}") for i in range(NCH)]
            kt = [res.tile([128, T], bf16, tag=f"kt{i}", name=f"kt{i}") for i in range(NCH)]
            qt = [res.tile([128, T], bf16, tag=f"qt{i}", name=f"qt{i}") for i in range(NCH)]
            yt = [res.tile([128, T], bf16, tag=f"yt{i}", name=f"yt{i}") for i in range(NCH)]
            # V tiles per 256-token block: [kpos-in-128, j(which 128), head, d+1]
            vt = [res.tile([128, 2, NH, HD + 1], pdt, tag=f"vt{i}", name=f"vt{i}")
                  for i in range(4)]
            osb = [res.tile([128, C], bf16, tag=f"osb{i}", name=f"osb{i}") for i in range(NT)]
            ytb = [res.tile([128, T], bf16, tag=f"ytb{i}", name=f"ytb{i}") for i in range(NCH)]
            recb = res.tile([4, 1024], f32, tag="recb", name="recb")
            recb16 = res.tile([4, 1024], bf16, tag="recb16", name="recb16")
            if exp_bias != 0.0:
                ebias = res.tile([128, 1], f32, tag="ebias", name="ebias")
                nc.gpsimd.memset(ebias, exp_bias)
            else:
                ebias = 0.0

            if FP8_QKV:
                for i in range(4):
                    nc.sync.dma_start(out=xt[i], in_=xTD[i, :, :, :])
            else:
                for i in range(NCH):
                    nc.scalar.dma_start(out=xt[i], in_=xTD[i * 128 : (i + 1) * 128, :])
            for tt2 in range(4):
                nc.gpsimd.memset(vt[tt2][:, :, :, HD : HD + 1], 1.0)

            def load_wp(g, cc, eng=None):
                wps = []
                for ftl in range(NCH):
                    wpt = wp_pool.tile([128, 512], bf16, tag="wp",
                                       name=f"wp{g}_{cc}_{ftl}")
                    (eng or nc.sync).dma_start(
                        out=wpt,
                        in_=wpTD[
                            g * C + ftl * 128 : g * C + (ftl + 1) * 128,
                            cc * 512 : (cc + 1) * 512,
                        ],
                    )
                    wps.append(wpt)
                return wps

            def load_wa(fbase, tag):
                """Queue DMA loads of W_attn.T columns fbase..fbase+1024."""
                groups = []
                for fg in range(2):  # 512-wide feature groups
                    f0 = fbase + fg * 512
                    was = []
                    if FP8_QKV:
                        for ct2 in range(4):
                            w = wa_pool.tile([128, 2, 512], fp8, tag="wa",
                                             name=f"wa_{tag}_{fg}_{ct2}")
                            nc.sync.dma_start(out=w, in_=waTD[ct2, :, :, f0 : f0 + 512])
                            was.append(w)
                    else:
                        for ct in range(NCH):
                            w = wa_pool.tile([128, 512], bf16, tag="wa",
                                             name=f"wa_{tag}_{fg}_{ct}")
                            nc.sync.dma_start(
                                out=w, in_=waTD[ct * 128 : (ct + 1) * 128, f0 : f0 + 512]
                            )
                            was.append(w)
                    groups.append(was)
                return groups

            def project_T(dst, fbase, tag):
                """dst[i][f_local, t] = (x @ W_attn.T).T rows fbase+0..1024,
                loading each 512-wide weight group just before its use."""
                for fg in range(2):  # 512-wide feature groups
                    f0 = fbase + fg * 512
                    was = []
                    for ct in range(NCH):
                        w = wa_pool.tile([128, 512], bf16, tag="wa",
                                         name=f"wa_{tag}_{fg}_{ct}")
                        nc.sync.dma_start(
                            out=w, in_=waTD[ct * 128 : (ct + 1) * 128, f0 : f0 + 512]
                        )
                        was.append(w)
                    for ftl in range(4):
                        fi = fg * 4 + ftl
                        ps = spq_pool.tile([128, 1024], f32, tag="spq",
                                           name=f"ps_{tag}_{fi}")
                        for tc2 in range(2):
                            if FP8_QKV:
                                for ct2 in range(4):
                                    nc.tensor.matmul(
                                        ps[:, tc2 * 512 : (tc2 + 1) * 512],
                                        was[ct2][:, :, ftl * 128 : (ftl + 1) * 128],
                                        xt[ct2][:, :, tc2 * 512 : (tc2 + 1) * 512],
                                        start=(ct2 == 0),
                                        stop=(ct2 == 3),
                                        perf_mode=DR,
                                    )
                            else:
                                for ct in range(NCH):
                                    nc.tensor.matmul(
                                        ps[:, tc2 * 512 : (tc2 + 1) * 512],
                                        was[ct][:, ftl * 128 : (ftl + 1) * 128],
                                        xt[ct][:, tc2 * 512 : (tc2 + 1) * 512],
                                        start=(ct == 0),
                                        stop=(ct == NCH - 1),
                                    )
                        nc.vector.tensor_copy(dst[fi], ps)

            # V in [token, feature] layout, ones column appended per head
            for fg in range(2):
                f0 = VOFF + fg * 512
                was = []
                if FP8_QKV:
                    for ct2 in range(4):
                        w = wa_pool.tile([128, 2, 512], fp8, tag="wa",
                                         name=f"wav_{fg}_{ct2}")
                        nc.sync.dma_start(out=w, in_=waTD[ct2, :, :, f0 : f0 + 512])
                        was.append(w)
                else:
                    for ct in range(NCH):
                        w = wa_pool.tile([128, 512], bf16, tag="wa",
                                         name=f"wav_{fg}_{ct}")
                        nc.scalar.dma_start(
                            out=w, in_=waTD[ct * 128 : (ct + 1) * 128, f0 : f0 + 512]
                        )
                        was.append(w)
                for tp in range(4):
                    ps = spq_pool.tile([128, 1024], f32, tag="spq", name=f"psv_{fg}_{tp}")
                    for half in range(2):
                        tt = tp * 2 + half
                        if FP8_QKV:
                            for ct2 in range(4):
                                nc.tensor.matmul(
                                    ps[:, half * 512 : (half + 1) * 512],
                                    xt[ct2][:, :, tt * 128 : (tt + 1) * 128],
                                    was[ct2],
                                    start=(ct2 == 0),
                                    stop=(ct2 == 3),
                                    perf_mode=DR,
                                )
                        else:
                            for ct in range(NCH):
                                nc.tensor.matmul(
                                    ps[:, half * 512 : (half + 1) * 512],
                                    xt[ct][:, tt * 128 : (tt + 1) * 128],
                                    was[ct],
                                    start=(ct == 0),
                                    stop=(ct == NCH - 1),
                                )
                    for half in range(2):
                        nc.vector.tensor_copy(
                            vt[tp][:, half, fg * 8 : (fg + 1) * 8, 0:HD],
                            ps[:, half * 512 : (half + 1) * 512].rearrange(
                                "p (a b) -> p a b", b=HD
                            ),
                        )

            def qproj_closures(dst, wa_groups, fi, tag):
                """One 128-feature tile of a Q projection as a list of
                single-matmul closures for fine-grained PE interleaving."""
                fg, ftl = fi // 4, fi % 4
                was = wa_groups[fg]
                cells = [{}, {}]

                def mk(i):
                    def go():
                        tc2, ct = divmod(i, NCH)
                        cell = cells[tc2]
                        if "ps" not in cell:
                            cell["ps"] = spq_pool.tile(
                                [128, 512], f32, tag="spf", bufs=2,
                                name=f"qf_{tag}_{fi}_{tc2}",
                            )
                        nc.tensor.matmul(
                            cell["ps"],
                            was[ct][:, ftl * 128 : (ftl + 1) * 128],
                            xt[ct][:, tc2 * 512 : (tc2 + 1) * 512],
                            start=(ct == 0),
                            stop=(ct == NCH - 1),
                        )
                    return go

                def mkfin(tc2):
                    def fin():
                        nc.vector.tensor_copy(
                            dst[fi][:, tc2 * 512 : (tc2 + 1) * 512],
                            cells[tc2]["ps"],
                        )
                    return fin

                mms = [mk(i) for i in range(8)] + [mkfin(0)] \
                    + [mk(i) for i in range(8, 16)] + [mkfin(1)]
                return mms, mms[-1]

            def proj_closures(gsrc, ysrc, wps_cell, cc, tp, f0=0, f1=NCH,
                              force_add=False):
                """One (cc, tp) piece of an output projection (contraction
                over ftl in [f0, f1)) as closures."""
                cells = [{}, {}]
                nf = f1 - f0

                def mk(i):
                    def go():
                        half, fo = divmod(i, nf)
                        ftl = f0 + fo
                        cell = cells[half]
                        if "ps" not in cell:
                            cell["ps"] = spq_pool.tile(
                                [128, 512], f32, tag="spf", bufs=2,
                                name=f"psp{gsrc}_{cc}_{tp}_{half}_{f0}",
                            )
                        tt = tp * 2 + half
                        nc.tensor.matmul(
                            cell["ps"],
                            ysrc[ftl][:, tt * 128 : (tt + 1) * 128],
                            wps_cell[cc][ftl],
                            start=(ftl == f0),
                            stop=(ftl == f1 - 1),
                        )
                    return go

                def mkfin(half):
                    def fin():
                        tt = tp * 2 + half
                        dst = osb[tt][:, cc * 512 : (cc + 1) * 512]
                        src = cells[half]["ps"]
                        if gsrc == 0 and not force_add:
                            nc.vector.tensor_copy(dst, src)
                        else:
                            nc.vector.tensor_add(dst, dst, src)
                    return fin

                return [mk(i) for i in range(nf)] + [mkfin(0)] \
                    + [mk(i) for i in range(nf, 2 * nf)] + [mkfin(1)]

            def proj_tile(gsrc, ysrc, wps, cc, tp):
                """One (cc, tp) piece of an output projection (PE filler)."""
                ps = spq_pool.tile([128, 1024], f32, tag="spq",
                                   name=f"psp{gsrc}_{cc}_{tp}")
                for half in range(2):
                    tt = tp * 2 + half
                    for ftl in range(NCH):
                        nc.tensor.matmul(
                            ps[:, half * 512 : (half + 1) * 512],
                            ysrc[ftl][:, tt * 128 : (tt + 1) * 128],
                            wps[ftl],
                            start=(ftl == 0),
                            stop=(ftl == NCH - 1),
                        )
                for half in range(2):
                    tt = tp * 2 + half
                    dst = osb[tt][:, cc * 512 : (cc + 1) * 512]
                    src = ps[:, half * 512 : (half + 1) * 512]
                    if gsrc == 0:
                        nc.vector.tensor_copy(dst, src)
                    else:
                        nc.vector.tensor_add(dst, dst, src)

            yts_all = [yt, ytb]
            # Q(set 0) fully up front; K f-tile 0 up front, K f-tiles 1-7
            # interleaved into set 0's attention as filler work
            project_T(qt, 0, "q0")
            ka_groups = load_wa(KOFF, "k")
            k0_mms, _ = qproj_closures(kt, ka_groups, 0, "k")
            for fn in k0_mms:
                fn()
            self_wa = None  # Q weight groups for the current set (sets 1-3)

            for g in range(NQS):
                ycur = yts_all[g % 2]
                yprev = yts_all[(g - 1) % 2]
                wps_cc = [load_wp(g - 1, 0), None] if g > 0 else None
                wps_last = [None, None]
                ysts = {}
                for hp in range(NHP):
                    # PE filler closures: one f-tile of next set's Q projection
                    # + one piece of the previous set's output projection,
                    # emitted interleaved between attention matmuls so they
                    # sit AHEAD of stall-prone instructions in the PE queue
                    fillers = []
                    if g == 0 and hp + 1 < NHP:
                        mmsk, _ = qproj_closures(kt, ka_groups, hp + 1, "k")
                        fillers += mmsk
                    if self_wa is not None and hp + 1 < NHP:
                        mmsq, _ = qproj_closures(qt, self_wa, hp + 1,
                                                 f"q{g}")
                        fillers += mmsq
                    if wps_cc is not None:
                        if g < NQS - 1:
                            if hp == 3:
                                wps_cc[1] = load_wp(g - 1, 1)
                            fillers += proj_closures(g - 1, yprev, wps_cc,
                                                     hp // 4, hp % 4)
                        else:
                            # last set: drain proj(g-1) early (2 pieces per
                            # block over hp0-3), then prefetch this set's wp
                            # and run ftl 0-5 of its projection in hp6/7
                            if hp == 1:
                                wps_cc[1] = load_wp(g - 1, 1)
                            if hp < 4:
                                for pi in (2 * hp, 2 * hp + 1):
                                    fillers += proj_closures(
                                        g - 1, yprev, wps_cc,
                                        pi // 4, pi % 4)
                            if hp == 4:
                                wps_last[0] = load_wp(g, 0, eng=nc.scalar)
                            if hp == 5:
                                wps_last[1] = load_wp(g, 1, eng=nc.scalar)
                            if hp >= 6:
                                cc = hp - 6
                                for tp in range(4):
                                    fillers += proj_closures(
                                        g, ycur, wps_last, cc, tp,
                                        f0=0, f1=6, force_add=True)

                    def fill(n):
                        for _ in range(n):
                            if fillers:
                                fillers.pop(0)()

                    for qc in range(2):
                        yp = yp_pool.tile([65, 1024], f32, tag="yp",
                                          name=f"yp{g}_{hp}_{qc}")
                        kbs = list(range(2 * qc + 2))
                        prev = None

                        def emit_av(kb, qlo, w, qoff):
                            first = kb == 0
                            last = kb == kbs[-1]
                            if FP8_AV:
                                for hh in range(2):
                                    nc.tensor.matmul(
                                        yp[0:65, hh * 512 + qoff : hh * 512 + qoff + w],
                                        vt[kb][:, :, 2 * hp + hh, :],
                                        pts[kb][:, hh, :, qoff : qoff + w],
                                        start=first,
                                        stop=last,
                                        perf_mode=DR,
                                    )
                            else:
                                for j in range(2):
                                    k2 = 2 * kb + j
                                    qlo_av = max(qc * 512, k2 * 128)
                                    w_av = qc * 512 + 512 - qlo_av
                                    qo_av = qlo_av - qc * 512
                                    for hh in range(2):
                                        nc.tensor.matmul(
                                            yp[0:65,
                                               hh * 512 + qo_av : hh * 512 + qo_av + w_av],
                                            vt[kb][:, j, 2 * hp + hh, :],
                                            pts[kb][:, hh, j, qo_av : qo_av + w_av],
                                            start=(first and j == 0),
                                            stop=(last and j == 1),
                                        )

                        pts = {}
                        geom = {}
                        for kb in kbs:
                            geom[kb] = (0, 0, 0)
                            ptile = pt_pool.tile([128, 2, 2, 512], pdt, tag="pt",
                                                 name=f"pt{g}_{hp}_{qc}_{kb}")
                            pts[kb] = ptile
                            for j in range(2):
                                k2 = 2 * kb + j
                                qlo = max(qc * 512, k2 * 128)
                                w = qc * 512 + 512 - qlo
                                qoff = qlo - qc * 512
                                sp = spq_pool.tile([128, 1024], f32, tag="spq",
                                                   name=f"sp{g}_{hp}_{qc}_{k2}")
                                for hh in range(2):
                                    ro = hh * 64
                                    nc.tensor.matmul(
                                        sp[:, hh * 512 : hh * 512 + w],
                                        kt[hp][ro : ro + 64, k2 * 128 : (k2 + 1) * 128],
                                        qt[hp][ro : ro + 64, qlo : qlo + w],
                                        start=True,
                                        stop=True,
                                    )
                                nc.scalar.activation(
                                    ptile[:, :, j, qoff : qoff + w],
                                    sp.rearrange("p (h q) -> p h q", h=2)[:, :, 0:w],
                                    EXP,
                                    bias=ebias,
                                    scale=qk_scale,
                                )
                                if k2 * 128 >= qc * 512:
                                    # zero the diagonal triangle (k > q) on gpsimd
                                    nc.gpsimd.affine_select(
                                        out=ptile[:, :, j, qoff : qoff + 128],
                                        in_=ptile[:, :, j, qoff : qoff + 128],
                                        compare_op=mybir.AluOpType.is_ge,
                                        fill=0.0,
                                        base=0,
                                        pattern=[[0, 2], [1, 128]],
                                        channel_multiplier=-1,
                                    )
                                fill(2)
                            if prev is not None:
                                emit_av(prev, *geom[prev])
                                fill(1)
                            prev = kb
                        emit_av(prev, *geom[prev])
                        fill(1)

                        # unnormalized y (+ denominator row 64) -> bf16 staging
                        r = hp * 2 + qc
                        yst = yst_pool.tile([65, 1024], bf16, tag="yst",
                                            name=f"yst{g}_{hp}_{qc}")
                        ysts[r] = yst
                        nc.vector.tensor_copy(yst, yp)
                        rr = (hp % 2) * 2 + qc
                        if rr == 0:
                            denb = bc_pool.tile([4, 1024], bf16, tag="denb",
                                                name=f"denb{g}_{hp // 2}")
                        nc.sync.dma_start(
                            out=denb[rr : rr + 1, :], in_=yst[64:65, :]
                        )

                    # drain remaining filler work for this head pair
                    while fillers:
                        fillers.pop(0)()

                    # batched reciprocal + broadcast + normalize, per 2 head
                    # pairs (bounds yst liveness and spreads the DMA burst)
                    if hp % 2 == 1:
                        batch = g * 4 + hp // 2
                        rbase = batch * 4
                        den32 = bc_pool.tile([4, 1024], f32, tag="den32",
                                             name=f"den32_{batch}")
                        nc.vector.tensor_copy(den32, denb)
                        nc.vector.reciprocal_approx_fast(out=recb, in_=den32)
                        nc.vector.tensor_copy(recb16, recb)
                        nc.sync.dma_start(
                            out=rscratch[rbase : rbase + 4, :], in_=recb16
                        )
                        for hp2 in range(hp - 1, hp + 1):
                            for qc2 in range(2):
                                r = hp2 * 2 + qc2
                                rr = (hp2 % 2) * 2 + qc2
                                yst = ysts.pop(r)
                                bcst = bc_pool.tile([64, 1024], bf16, tag="bc",
                                                    name=f"bc{g}_{hp2}_{qc2}")
                                rrow = rscratch[rbase + rr : rbase + rr + 1, :]
                                nc.sync.dma_start(
                                    out=bcst,
                                    in_=bass.AP(
                                        tensor=rrow.tensor,
                                        offset=rrow.offset,
                                        ap=[[0, 64]] + rrow.ap[1:],
                                    ),
                                )
                                nc.vector.tensor_mul(
                                    ycur[hp2][0:64, qc2 * 512 : (qc2 + 1) * 512],
                                    yst[0:64, 0:512],
                                    bcst[:, 0:512],
                                )
                                nc.vector.tensor_mul(
                                    ycur[hp2][64:128, qc2 * 512 : (qc2 + 1) * 512],
                                    yst[0:64, 512:1024],
                                    bcst[:, 512:1024],
                                )

                # load next set's Q weights and emit its f-tile 0 here, in
                # the set-boundary window
                if g + 1 < NQS:
                    self_wa = load_wa((g + 1) * C, f"q{g + 1}")
                    mms0, _ = qproj_closures(qt, self_wa, 0, f"q{g + 1}")
                    for fn in mms0:
                        fn()

            # last-set projection tail: only ftl 6-7 remain; out-DMA each
            # token tile as soon as both column halves have accumulated
            g = NQS - 1
            for tp in range(4):
                for cc in range(2):
                    for fn in proj_closures(g, yts_all[g % 2], wps_last, cc,
                                            tp, f0=6, f1=NCH, force_add=True):
                        fn()
                for half in range(2):
                    tt = tp * 2 + half
                    nc.sync.dma_start(
                        out=out[tt * 128 : (tt + 1) * 128, :], in_=osb[tt]
                    )

    nc.compile()
    return nc


def kernel(x, W_attn, W_proj, _trace=False):
    if "nc" not in _CACHE:
        _CACHE["nc"] = _build()
    nc = _CACHE["nc"]

    xT = np.ascontiguousarray(np.transpose(np.asarray(x, np.float32), (0, 2, 1)))
    waT = np.ascontiguousarray(np.asarray(W_attn, np.float32).T)
    wpT = np.ascontiguousarray(np.asarray(W_proj, np.float32).T)
    pdt = FP8 if FP8_AV else BF16
    if FP8_QKV:
        # interleave contraction dim: [C, N] -> [4, 128, 2, N], scaled for fp8
        xTs = np.clip(xT * XS, -240, 240).reshape(B, 4, 2, 128, T)
        xTs = np.ascontiguousarray(xTs.transpose(0, 1, 3, 2, 4)).astype(FP8)
        waTs = np.clip(waT * WS, -240, 240).reshape(4, 2, 128, 6 * C)
        waTs = np.ascontiguousarray(waTs.transpose(0, 2, 1, 3)).astype(FP8)
        wpTh = (wpT / (XS * WS)).astype(BF16)
        xs = [xTs[b] for b in range(B)]
    else:
        xs = [xT[b].astype(BF16) for b in range(B)]
        waTs = waT.astype(BF16)
        wpTh = wpT.astype(BF16)
    in_maps = [
        {"xT": xs[b], "waT": waTs, "wpT": wpTh} for b in range(B)
    ]
    res = run_bass_kernel_spmd(nc, in_maps, core_ids=list(range(B)), trace=_trace)
    LAST["exec_time_ns"] = res.exec_time_ns
    LAST["mean_exec_time_ns"] = res.mean_exec_time_ns
    LAST["results"] = res
    return np.stack([res.results[b]["out"] for b in range(B)]).astype(np.float32)
